# revision 23
# baseline (speedup 1.0000x reference)
"""Trainium2 Bass kernel for nn_MultiHeadAttention_67018669687091.

Problem: MHA with B=2, S=2048, E=1024, H=16, D=64, causal, fp32.
The reference reshapes (B,S,E)->(B,H,S,D) WITHOUT transpose, so head h of
batch b is the contiguous 128-row x-block rows [h*128,(h+1)*128) viewed as a
(2048, 64) pseudo-sequence: position 16*s+j <- (row s, channel 64j+d).

Sharding: 8 cores; core c owns batch b=c//4 and head-quad qd=c%4 (4 heads).
Each core computes the qkv projection for its 4 blocks, per-head causal
attention in the transposed domain (scores with key on partitions, softmax
along the free dim via an augmented ones-row of V and late normalization),
and a row-parallel slice of the output projection. Host sums the 4 partials
per batch and adds bout.

Matmul operand dtype is selectable: bfloat16 (fastest), float32r, float32.
"""
import numpy as np
from contextlib import ExitStack

import concourse.bass as bass
import concourse.bacc as bacc
import concourse.mybir as mybir
import concourse.tile as tile
from concourse.masks import make_identity
from concourse.bass_utils import run_bass_kernel_spmd

E = 1024
H = 16
D = 64
B = 2
S = 2048
HPC = 4          # heads per core
SL = HPC * 128   # x columns per core (512)

F32 = mybir.dt.float32
F32R = mybir.dt.float32r
BF16 = mybir.dt.bfloat16
EXP = mybir.ActivationFunctionType.Exp


def _pieces(lo, hi, bank=512):
    """Split [lo, hi) at multiples of `bank` (PSUM bank boundaries)."""
    out = []
    while lo < hi:
        nxt = min(hi, (lo // bank + 1) * bank)
        out.append((lo, nxt))
        lo = nxt
    return out


def build_program(mm_dt=BF16):
    """One SPMD program; per-core data comes via in_maps."""
    nc = bacc.Bacc("TRN2", target_bir_lowering=False)
    DT = mm_dt
    # dtype of the qkv tiles / transposes: must pair legally with identity
    QKVDT = DT if DT == BF16 else F32

    xT = nc.dram_tensor("xT", [E, SL], DT, kind="ExternalInput").ap()
    wqkvT = nc.dram_tensor("wqkvT", [E, 3 * E], DT, kind="ExternalInput").ap()
    bqkv = nc.dram_tensor("bqkv", [1, 3 * E], F32, kind="ExternalInput").ap()
    woutT = nc.dram_tensor("woutT", [2 * 128, E], DT, kind="ExternalInput").ap()
    outT = nc.dram_tensor("partialT", [E, S], F32, kind="ExternalOutput").ap()

    with tile.TileContext(nc) as tc, ExitStack() as ctx:
        singles = ctx.enter_context(tc.tile_pool(name="singles", bufs=1))
        wpool = ctx.enter_context(tc.tile_pool(name="wpool", bufs=2))
        qkvpool = ctx.enter_context(tc.tile_pool(name="qkvpool", bufs=1))
        headpool = ctx.enter_context(tc.tile_pool(name="headpool", bufs=3))
        ppool = ctx.enter_context(tc.tile_pool(name="ppool", bufs=3))
        cpool = ctx.enter_context(tc.tile_pool(name="cpool", bufs=1))
        stpool = ctx.enter_context(tc.tile_pool(name="stpool", bufs=3))
        small = ctx.enter_context(tc.tile_pool(name="small", bufs=2))
        dpool = ctx.enter_context(tc.tile_pool(name="dpool", bufs=2, space="DRAM"))
        mmps = ctx.enter_context(tc.tile_pool(name="mmps", bufs=2, space="PSUM"))
        sps = ctx.enter_context(tc.tile_pool(name="sps", bufs=2, space="PSUM"))
        ops = ctx.enter_context(tc.tile_pool(name="ops", bufs=1, space="PSUM"))

        ident = singles.tile([128, 128], QKVDT, tag="ident")
        make_identity(nc, ident)

        # PE warmup: ~7us of back-to-back matmuls on scratch data raises the
        # HAM clock gate to 8/8 before the real work arrives (and fills the
        # initial input-DMA wait).
        wsrc = singles.tile([128, 512], DT, tag="wsrc")
        nc.vector.memset(wsrc, 0.0)
        wps = mmps.tile([128, 512], F32, tag="mm")
        for i in range(32):
            nc.tensor.matmul(wps, lhsT=wsrc[:, 0:128], rhs=wsrc,
                             start=(i == 0), stop=(i == 31))

        # mask01[p, f] = 1 if p <= f else 0  (key <= query inside the
        # diagonal 128-chunk of the transposed score tile)
        mask01 = singles.tile([128, 128], BF16, tag="mask01")
        nc.vector.memset(mask01, 1.0)
        nc.gpsimd.affine_select(
            out=mask01,
            in_=mask01,
            compare_op=mybir.AluOpType.is_ge,
            fill=0.0,
            base=0,
            pattern=[[1, 128]],
            channel_multiplier=-1,
        )

        bias_sb = singles.tile([128, 3 * E], F32, tag="bias")
        bq_bcast = bass.AP(
            tensor=bqkv.tensor, offset=bqkv.offset,
            ap=[[0, 128]] + [list(d) for d in bqkv.ap[1:]],
        )
        nc.sync.dma_start(out=bias_sb, in_=bq_bcast)

        wout_sb = singles.tile([128, 2, E], DT, tag="wout")
        nc.sync.dma_start(out=wout_sb, in_=woutT.rearrange("(hf p) j -> p hf j", p=128))

        xts = []
        for hl in range(HPC):
            xt = singles.tile([128, 8, 128], DT, tag=f"xt{hl}")
            nc.sync.dma_start(
                out=xt,
                in_=xT.rearrange("(ec p) s -> p ec s", p=128)[:, :, hl * 128:(hl + 1) * 128],
            )
            xts.append(xt)

        # ---- QKV projection: qkv[hl] = x_blk @ WqkvT + bqkv  (128, 3072)
        qkvs = [
            qkvpool.tile([128, 3 * E], QKVDT, tag=f"qkv{hl}", name=f"qkv{hl}")
            for hl in range(HPC)
        ]
        wq3 = wqkvT.rearrange("(ec p) c -> p ec c", p=128)

        # ---- per-head attention prep machinery. qkv columns of tensor t
        # (q/k/v) are exactly proj nb-blocks (2t, 2t+1), so each tensor's
        # transposes + scatter copies are emitted right after its two proj
        # blocks: the DVE scatter stream for heads 0/1 runs during the
        # projection (where DVE is otherwise idle) instead of after it.
        conA = cpool.tile([128, S], DT, tag="conA")
        conB = cpool.tile([128, S], DT, tag="conB")
        preps = {}

        def head_tiles(hl):
            if hl not in preps:
                # QT/KT zero-padded to 128 partitions: K=128 full-array S
                # matmuls stream at 216ns (K=64 serializes LDWEIGHTS).
                QT = headpool.tile([128, S], DT, tag="QT", name=f"QT{hl}")
                KT = headpool.tile([128, S], DT, tag="KT", name=f"KT{hl}")
                VT = headpool.tile([65, S], QKVDT, tag="VT", name=f"VT{hl}")
                Vc = headpool.tile([128, 16, 65], DT, tag="Vc", name=f"Vc{hl}")
                nc.gpsimd.memset(QT[64:128, :], 0.0)
                nc.gpsimd.memset(KT[64:128, :], 0.0)
                nc.gpsimd.memset(VT[64:65, :], 1.0)
                preps[hl] = [QT, KT, VT, Vc]
            return preps[hl]

        def emit_tensor_prep(hl, t):
            dest = head_tiles(hl)[t]
            tpb = mmps.tile([128, 1024], QKVDT, tag="mm", space="PSUM",
                            name=f"tpb{hl}_{t}")
            for cc in range(8):
                nc.tensor.transpose(
                    tpb[:, cc * 128:(cc + 1) * 128],
                    qkvs[hl][:, t * 1024 + cc * 128: t * 1024 + (cc + 1) * 128],
                    ident,
                )
            t3 = tpb.rearrange("p (cc s) -> p cc s", cc=8)
            for jp in range(2):
                csrc = t3[64 * jp:64 * jp + 64, :, :]
                dd = dest[0:64].rearrange("d (s jc j2) -> d jc j2 s", jc=8, j2=2)[:, :, jp, :]
                nc.vector.tensor_copy(dd, csrc)

        def emit_vc_prep(hl):
            QT, KT, VT, Vc = head_tiles(hl)
            for kc in range(16):
                tp = mmps.tile([128, 512], QKVDT, tag="mm", space="PSUM",
                               name=f"tpv{hl}_{kc}")
                nc.tensor.transpose(tp[:, 0:65], VT[:, kc * 128:(kc + 1) * 128], ident[0:65, 0:65])
                nc.vector.tensor_copy(Vc[:, kc, :], tp[:, 0:65])

        def emit_prep(hl):
            for t in range(3):
                emit_tensor_prep(hl, t)
            emit_vc_prep(hl)

        # ---- QKV projection, interleaved with heads 0/1 prep
        for t in range(3):
            for nb in (2 * t, 2 * t + 1):
                wts = []
                for ec in range(8):
                    wtc = wpool.tile([128, 512], DT, tag=f"wt{ec}", name=f"wt{nb}_{ec}")
                    nc.sync.dma_start(
                        out=wtc, in_=wq3[:, ec, nb * 512:(nb + 1) * 512],
                    )
                    wts.append(wtc)
                for hl in range(HPC):
                    ps = mmps.tile([128, 512], F32, tag="mm")
                    for ec in range(8):
                        nc.tensor.matmul(
                            ps, lhsT=xts[hl][:, ec, :], rhs=wts[ec],
                            start=(ec == 0), stop=(ec == 7),
                        )
                    nc.vector.tensor_add(
                        qkvs[hl][:, nb * 512:(nb + 1) * 512], ps,
                        bias_sb[:, nb * 512:(nb + 1) * 512],
                    )
            for hl in (0, 1):
                emit_tensor_prep(hl, t)
        for hl in (0, 1):
            emit_vc_prep(hl)

        def emit_attention(hl):
            QT, KT, VT, Vc = preps.pop(hl)
            con = conA if hl < 2 else conB
            r0 = 64 * (hl % 2)
            for qh in range(2):
                outp = ops.tile([65, 1024], F32, tag="outp", space="PSUM",
                                name=f"outp{hl}_{qh}")
                for kc in range(8 * (qh + 1)):
                    qstart = max(kc * 128, qh * 1024)
                    qlen = (qh + 1) * 1024 - qstart
                    St = sps.tile([128, 1024], F32, tag="S", space="PSUM",
                                  name=f"St{hl}_{qh}_{kc}")
                    for (a, b) in _pieces(0, qlen):
                        nc.tensor.matmul(
                            St[:, a:b],
                            lhsT=KT[:, kc * 128:(kc + 1) * 128],
                            rhs=QT[:, qstart + a: qstart + b],
                            start=True, stop=True,
                        )
                    P = ppool.tile([128, 1024], DT, tag="P", name=f"P{hl}_{qh}_{kc}")
                    nc.scalar.activation(P[:, 0:qlen], St[:, 0:qlen], EXP, scale=0.125)
                    if kc * 128 == qstart:
                        nc.vector.tensor_mul(P[:, 0:128], P[:, 0:128], mask01)
                    rel = qstart - qh * 1024
                    for (a, b) in _pieces(rel, rel + qlen):
                        last_kc = 8 * qh + 4 * (a // 512) + 3
                        nc.tensor.matmul(
                            outp[:, a:b],
                            lhsT=Vc[:, kc, :],
                            rhs=P[:, a - rel: b - rel],
                            start=(kc == 0), stop=(kc == last_kc),
                        )
                # stage PV out of PSUM on ACT (frees the banks; keeps the
                # DVE queue clear for the prep scatter stream)
                stg = small.tile([65, 1024], F32, tag="stg", name=f"stg{hl}_{qh}")
                nc.vector.tensor_copy(stg, outp)
                # reciprocal of the denominator row, partition-spread via a
                # DRAM round-trip (a (1,1024) single-lane DVE reciprocal
                # costs 6.5us; spread across 128 partitions it is ~60ns).
                d_dram = dpool.tile([1, 1024], F32, tag="d_dram", name=f"dd{hl}_{qh}")
                nc.sync.dma_start(out=d_dram, in_=stg[64:65, :])
                spread = small.tile([128, 8], F32, tag="spread", name=f"sp{hl}_{qh}")
                nc.sync.dma_start(
                    out=spread,
                    in_=d_dram.rearrange("a (p i) -> p a i", p=128)[:, 0, :],
                )
                rspread = small.tile([128, 8], F32, tag="rspread", name=f"rs{hl}_{qh}")
                nc.vector.reciprocal(rspread, spread)
                r_dram = dpool.tile([1, 1024], F32, tag="r_dram", name=f"rd{hl}_{qh}")
                nc.sync.dma_start(
                    out=r_dram.rearrange("a (p i) -> p a i", p=128)[:, 0, :],
                    in_=rspread,
                )
                rec64 = small.tile([64, 1024], F32, tag="rec64", name=f"r64{hl}_{qh}")
                rec_bcast = bass.AP(
                    tensor=r_dram.tensor, offset=r_dram.offset,
                    ap=[[0, 64]] + [list(d) for d in r_dram.ap[1:]],
                )
                nc.sync.dma_start(out=rec64, in_=rec_bcast)
                nc.vector.tensor_mul(
                    con[r0:r0 + 64, qh * 1024:(qh + 1) * 1024],
                    stg[0:64, :], rec64,
                )

        emit_attention(0)
        emit_prep(2)
        emit_attention(1)
        emit_prep(3)
        emit_attention(2)
        emit_attention(3)

        # ---- output projection: partialT[j, s] = woutT_s.T @ [conA; conB]
        for jc in range(8):
            for sb in range(4):
                ps = mmps.tile([128, 512], F32, tag="mm")
                nc.tensor.matmul(
                    ps, lhsT=wout_sb[:, 0, jc * 128:(jc + 1) * 128],
                    rhs=conA[:, sb * 512:(sb + 1) * 512],
                    start=True, stop=False,
                )
                nc.tensor.matmul(
                    ps, lhsT=wout_sb[:, 1, jc * 128:(jc + 1) * 128],
                    rhs=conB[:, sb * 512:(sb + 1) * 512],
                    start=False, stop=True,
                )
                st = stpool.tile([128, 512], F32, tag="st")
                nc.scalar.copy(st, ps)
                nc.sync.dma_start(
                    out=outT[jc * 128:(jc + 1) * 128, sb * 512:(sb + 1) * 512],
                    in_=st,
                )
    nc.compile()
    return nc


def make_in_maps(x, Wqkv, bqkv, Wout, mm_dt=BF16):
    np_dt = mybir.dt.np(mm_dt)
    x = np.asarray(x, np.float32)
    xT = np.ascontiguousarray(x.transpose(0, 2, 1)).astype(np_dt)  # (2,1024,2048)
    WqkvT = np.ascontiguousarray(np.asarray(Wqkv, np.float32).T).astype(np_dt)
    WoutT = np.ascontiguousarray(np.asarray(Wout, np.float32).T).astype(np_dt)
    bq = np.asarray(bqkv, np.float32).reshape(1, 3 * E)
    in_maps = []
    for c in range(8):
        b, qd = divmod(c, 4)
        in_maps.append({
            "xT": np.ascontiguousarray(xT[b][:, qd * SL:(qd + 1) * SL]),
            "wqkvT": WqkvT,
            "bqkv": bq,
            "woutT": np.ascontiguousarray(WoutT[qd * 256:(qd + 1) * 256, :]),
        })
    return in_maps


_NC_CACHE = {}


def get_program(mm_dt=BF16):
    key = str(mm_dt)
    if key not in _NC_CACHE:
        _NC_CACHE[key] = build_program(mm_dt)
    return _NC_CACHE[key]


def assemble(results, bout):
    bout = np.asarray(bout, np.float32)
    out = np.zeros((B, S, E), np.float32)
    for c in range(8):
        b = c // 4
        out[b] += results[c]["partialT"].T
    out += bout
    return out


def kernel(x, Wqkv, bqkv, Wout, bout, mm_dt=BF16, trace=False):
    nc = get_program(mm_dt)
    in_maps = make_in_maps(x, Wqkv, bqkv, Wout, mm_dt)
    res = run_bass_kernel_spmd(nc, in_maps, list(range(8)), trace=trace)
    out = assemble(res.results, bout)
    if trace:
        kernel.last_result = res
    return out
